# revision 2
# baseline (speedup 1.0000x reference)
"""Trainium2 Bass kernel for nn_ConvPolicy — fp16 I/O, DVE/ACT split.

Per row of x[B, 18] (fp16 on device, f32 at the host boundary):
  u    = relu(obs @ Wo.T + bo)            # [2]
  c1_t = relu(sum_k x[4+2t+k]*cw0k + x[11+2t+k]*cw1k + cb), t=0..2
  s_t  = relu(c1_t*c2w0 + c1_{t+1}*c2w1 + c2b), t=0,1
  e    = relu((u + s) @ We.T + be)        # [2]
  g0 = relu(e0*v0 + d1b); g1 = relu(e0*v1 + e1*v0 + d1b); g2 = relu(e1*v1 + d1b)
  y[2t+k] = sum_t g_t*w_k + b  (deconv2, 7 outputs row-major)

Design notes (from cost-model analysis):
  - fp16 end-to-end halves HBM traffic (gate is rel<2e-2; measured 8.7e-4).
  - MAC chains (scalar_tensor_tensor) run 1 el/cycle on DVE regardless of
    dtype/stride; first terms + relus + single-term affines go to ACT
    (func(scale*x+bias) fused, 1.2 GHz).
  - u+s runs as tensor_tensor add on DVE (2x_1p mode: packed fp16).
  - GpSimd/Pool supports no generic tensor ops on this toolchain.
  - Variable tile sizes: small first tile starts compute early, small last
    tile shortens the serial drain chain.
Weights are baked into the instruction stream as immediates at build time.
"""

import numpy as np

B = 2_000_000
N_CORES = 8
P = 128

# Variable tile sizes (rows per partition per tile).
C = [160, 400, 480, 480, 342, 100]
ROWS_PER_CORE = P * sum(C)         # 251_136
PADDED = ROWS_PER_CORE * N_CORES   # 2_009_088


def _build(weights: dict, c_list, reps: int = 1, mode: str = "full"):
    import concourse.bass as bass
    import concourse.mybir as mybir
    from concourse.tile import TileContext

    f16 = mybir.dt.float16
    MULT = mybir.AluOpType.mult
    ADD = mybir.AluOpType.add
    MAX = mybir.AluOpType.max
    RELU = mybir.ActivationFunctionType.Relu
    IDENT = mybir.ActivationFunctionType.Identity

    c_list = list(c_list)
    rows = P * sum(c_list)
    cmax = max(c_list)

    wo = weights["fc_obs_w"]          # [2, 4]
    bo = weights["fc_obs_b"]          # [2]
    we = weights["fc_emb_w"]          # [2, 2]
    be = weights["fc_emb_b"]          # [2]
    cw = weights["conv1_w"][0]        # [2, 3]
    cb = float(weights["conv1_b"][0])
    c2 = weights["conv2_w"][0, 0]     # [2]
    c2b = float(weights["conv2_b"][0])
    dv = weights["deconv1_w"][0, 0]   # [2]
    d1b = float(weights["deconv1_b"][0])
    dw = weights["deconv2_w"][0, 0]   # [3]
    d2b = float(weights["deconv2_b"][0])

    nc = bass.Bass()
    x = nc.declare_dram_parameter("x", [rows, 18], f16, isOutput=False)
    y = nc.declare_dram_parameter("y", [rows, 7], f16, isOutput=True)

    def xyv(t):
        base = P * sum(c_list[:t])
        ci = c_list[t]
        xs = x[base:base + P * ci].rearrange("(p c) f -> p (c f)", p=P, c=ci)
        ys = y[base:base + P * ci].rearrange("(p c) g -> p (c g)", p=P, c=ci)
        return xs, ys, ci

    def stt(out, in0, s, in1):
        # out = in0 * s + in1  (fused MAC on DVE)
        nc.vector.scalar_tensor_tensor(out=out, in0=in0, scalar=float(s),
                                       in1=in1, op0=MULT, op1=ADD)

    bias_vals = sorted(
        {float(v) for v in (bo[0], bo[1], cb, c2b, be[0], be[1], d1b, d2b)}
    )
    bias_ap = {}

    def aff(out, in_, s, b):
        # out = in_ * s + b  (ScalarE affine)
        nc.scalar.activation(out, in_, IDENT, bias=bias_ap[float(b)],
                             scale=float(s))

    def act_relu(out, in_, s=1.0, b=0.0):
        bias = bias_ap[float(b)] if b else 0.0
        nc.scalar.activation(out, in_, RELU, bias=bias, scale=float(s))

    with TileContext(nc) as tc:
        with (
            tc.tile_pool(name="const", bufs=1) as cpool,
            tc.tile_pool(name="xin", bufs=3) as xp,
            tc.tile_pool(name="yout", bufs=3) as ypool,
            tc.tile_pool(name="mid", bufs=3) as mp,
        ):
            btile = cpool.tile([P, len(bias_vals)], mybir.dt.float32)
            for i, v in enumerate(bias_vals):
                nc.vector.memset(btile[:, i:i + 1], v)
                bias_ap[v] = btile[:, i:i + 1]

            def body(t):
                xs, ys, c = xyv(t)
                xt = xp.tile([P, 18 * c], f16, tag="x")
                nc.sync.dma_start(out=xt[:], in_=xs)
                X = xt[:].rearrange("p (c f) -> p c f", f=18)   # [P, c, 18]
                X3 = xt[:].rearrange("p (c f) -> p f c", f=18)  # [P, 18, c]

                U = mp.tile([P, 2 * c], f16, tag="U")
                C1 = mp.tile([P, 3 * c], f16, tag="C1")
                S = mp.tile([P, 2 * c], f16, tag="S")
                E = mp.tile([P, 2 * c], f16, tag="E")
                D = mp.tile([P, 3 * c], f16, tag="D")
                Y = ypool.tile([P, 7 * c], f16, tag="y")

                if mode == "dma":
                    nc.sync.dma_start(out=ys, in_=xt[:, 0:7 * c])
                    return

                # --- fc_obs ---
                for ch in range(2):
                    dst = U[:, ch * c:(ch + 1) * c]
                    aff(dst, X[:, :, 0], wo[ch, 0], bo[ch])
                    for i in range(1, 4):
                        stt(dst, X[:, :, i], wo[ch, i], dst)

                # --- conv1 over [P,3,c] tap views ---
                C1v = C1[:].rearrange("p (t c) -> p t c", t=3)
                aff(C1v, X3[:, 4:10:2, :], cw[0, 0], cb)
                stt(C1v, X3[:, 5:11:2, :], cw[0, 1], C1v)
                stt(C1v, X3[:, 6:12:2, :], cw[0, 2], C1v)
                stt(C1v, X3[:, 11:17:2, :], cw[1, 0], C1v)
                stt(C1v, X3[:, 12:18:2, :], cw[1, 1], C1v)
                stt(C1v, X3[:, 13:18:2, :], cw[1, 2], C1v)

                act_relu(U[:], U[:])
                act_relu(C1[:], C1[:])

                # --- conv2 ---
                aff(S[:], C1[:, 0:2 * c], c2[0], c2b)
                stt(S[:], C1[:, c:3 * c], c2[1], S[:])
                act_relu(S[:], S[:])

                # --- u + s (DVE tensor_tensor add, 2x packed fp16) ---
                nc.vector.tensor_tensor(out=S[:], in0=U[:], in1=S[:], op=ADD)

                # --- fc_emb ---
                for ch in range(2):
                    dst = E[:, ch * c:(ch + 1) * c]
                    aff(dst, S[:, 0:c], we[ch, 0], be[ch])
                    stt(dst, S[:, c:2 * c], we[ch, 1], dst)
                act_relu(E[:], E[:])

                # --- deconv1 -> G (3 channels in D) ---
                act_relu(D[:, 0:c], E[:, 0:c], s=dv[0], b=d1b)              # g0
                act_relu(D[:, 2 * c:3 * c], E[:, c:2 * c], s=dv[1], b=d1b)  # g2
                aff(D[:, c:2 * c], E[:, 0:c], dv[1], d1b)
                stt(D[:, c:2 * c], E[:, c:2 * c], dv[0], D[:, c:2 * c])
                act_relu(D[:, c:2 * c], D[:, c:2 * c])

                # --- deconv2 -> Y row-major: y[2t+k] += g_t*w_k + b ---
                Yv = Y[:].rearrange("p (c g) -> p g c", g=7)  # [P, 7, c]
                Gv = D[:].rearrange("p (l c) -> p l c", l=3)  # [P, 3, c]
                aff(Yv[:, 0:5:2, :], Gv, dw[0], d2b)                 # y0,y2,y4
                stt(Yv[:, 2:6:2, :], Gv[:, 0:2, :], dw[2],
                    Yv[:, 2:6:2, :])                                 # y2,y4 +=
                aff(Yv[:, 1:6:2, :], Gv, dw[1], d2b)                 # y1,y3,y5
                aff(Yv[:, 6, :], D[:, 2 * c:3 * c], dw[2], d2b)      # y6

                nc.sync.dma_start(out=ys, in_=Y[:])

            if reps > 1:
                with tc.For_i(0, reps):
                    for t in range(len(c_list)):
                        body(t)
            else:
                for t in range(len(c_list)):
                    body(t)

    _split_multi_waits(nc)
    return nc


def _split_multi_waits(nc):
    """Walrus codegen accepts at most ONE sync-wait per instruction; hoist
    extra waits onto standalone same-engine NoOps placed just before."""
    import concourse.mybir as mybir

    n = 0
    for fn in nc.m.functions:
        for bb in fn.blocks:
            out = []
            for ins in bb.instructions:
                si = getattr(ins, "sync_info", None)
                waits = list(si.on_wait) if si and si.on_wait else []
                if len(waits) > 1:
                    for w in waits[:-1]:
                        nop = mybir.InstNoOp(name=f"waitnop-{n}", ins=[], outs=[])
                        n += 1
                        nop.engine = ins.engine
                        nop.sync_info = mybir.SyncInfo(on_wait=[w], on_update=[])
                        out.append(nop)
                    ins.sync_info = mybir.SyncInfo(
                        on_wait=[waits[-1]], on_update=list(si.on_update or [])
                    )
                out.append(ins)
            bb.instructions = out


LAST_RESULTS = None  # test harness introspection (exec_time_ns, profile)


def kernel(**inputs) -> np.ndarray:
    global LAST_RESULTS
    from concourse.bass_utils import run_bass_kernel_spmd

    x = np.asarray(inputs["x"], dtype=np.float32)
    weights = {
        k: np.asarray(v, dtype=np.float32) for k, v in inputs.items() if k != "x"
    }
    assert x.shape == (B, 18), x.shape

    nc = _build(weights, C)

    xp = np.zeros((PADDED, 18), dtype=np.float16)
    xp[:B] = x.astype(np.float16)
    shards = xp.reshape(N_CORES, ROWS_PER_CORE, 18)
    in_maps = [{"x": np.ascontiguousarray(shards[i])} for i in range(N_CORES)]

    LAST_RESULTS = run_bass_kernel_spmd(nc, in_maps, list(range(N_CORES)))
    outs = [np.asarray(LAST_RESULTS.results[i]["y"]) for i in range(N_CORES)]
    yf = np.concatenate(outs, axis=0)[:B].astype(np.float32)
    return np.ascontiguousarray(yf.reshape(B, 1, 7))
